# revision 56
# baseline (speedup 1.0000x reference)
"""MixGARCH Trainium2 kernel, v10 (final): B=8 block-scan, pipelined
fine-grained phase 2, DMA-instruction-lean (~56.2us vs 64.6us baseline).

Math (unchanged since v3): subtract the steady state s=(bias+1e-6)/(1-Wh)
so the recurrence is linear with no bias (ReLU is a no-op: all terms are
non-negative).  Per half (32768 steps + 256 warm-up): phase-1 matmuls
reduce each 8-step block to d_b = sum_i Wh^(7-i) Wx x2; a hardware
tensor_tensor_scan chains S_b = Wh^8 S_{b-1} + d_b for both halves at
once (128 partitions); phase-2 matmuls expand every block to its 8
outputs with the carry S_{b-1} folded into the same 128-deep contraction
(rhs rows 0:64 = packed x2, rows 64:128 = carry, placed by an SBUF->SBUF
partition-shift DMA).

Scheduling (what the traces drove):
  - every dma_start costs ~0.6us of issuing-engine time and ~2us of
    serialized ring completion latency, so DMA instructions are few and
    routed: cf (with winit + lhsT_dh bitcast-packed) then lhsT_p on the
    scalar ring; 6 merged input pieces (both halves via 3D rearrange
    APs) on sync; early-group S-copies on scalar, late on gpsimd; vout
    staged in one [128, 8*NB] tile and shipped as 2 merged 4-region
    DMAs per copy group (sync/gpsimd), 4 finer HWDGE DMAs in the last.
  - scan chunks [256,256,512x7,32]; phase-2 emitted per copy group
    ([512,1024,1024,1024,544]) region-major through 3 rotating PSUM
    tiles; PSUM->SBUF cast+bias copies split DVE/ACT to balance with
    the scan (~26us busy each, just under the ~27us HBM-DMA floor).
"""

import numpy as np
import ml_dtypes

BF16 = ml_dtypes.bfloat16

T = 524288
K = 64
NJ = 8
NCORES = 8
W = 256               # warmup steps per half
HALF = 32768
TT = W + HALF         # 33024 steps per half
B = 8                 # block size
NB = TT // B          # 4128 blocks per half
SCW = 512             # whB_wide width (max scan chunk)
# scan chunks (also d-matmul chunks)
SCHUNKS = [(0, 256), (256, 512), (512, 1024), (1024, 1536), (1536, 2048),
           (2048, 2560), (2560, 3072), (3072, 3584), (3584, 4096),
           (4096, 4128)]
# copy groups: spans of scan chunks that share one PSUM tile / copy / DMA
CGROUPS = [(0, 512), (512, 1536), (1536, 2560), (2560, 3584), (3584, 4128)]
# input DMA pieces (merged across both halves via 3D AP)
IPIECES = [(0, 256), (256, 512), (512, 1536), (1536, 2560), (2560, 3584),
           (3584, 4128)]

_CACHE = {}


def _weights_host(vars0, bias, Wx, Wh):
    Wx = Wx.astype(np.float64)
    Wh = Wh.astype(np.float64)
    bias = bias.astype(np.float64)
    s_steady = (bias + 1e-6) / (1.0 - Wh)
    whp = Wh[None, :] ** np.arange(10)[:, None]   # whp[e, k]
    vars0 = np.asarray(vars0, np.float64)

    # d/S layout partition: 64h + k.  xin rows (per half tile): 8i + l.
    # phase-2 out partition: 16j + kk (k = 16g + kk).
    lhsT_dh = np.zeros((64, 64), np.float64)
    for i in range(B):
        for l in range(NJ):
            for k in range(K):
                lhsT_dh[8 * i + l, k] = whp[7 - i, k] * Wx[k, l]

    # lhsT_p per g: [128, 128]; rows 0..64 intra-block, rows 64..128 carry.
    lhsT_p = np.zeros((128, 4 * 128), np.float64)
    for g in range(4):
        for j in range(B):
            for kk in range(16):
                k = 16 * g + kk
                col = 128 * g + 16 * j + kk
                for i in range(j + 1):
                    for l in range(NJ):
                        lhsT_p[8 * i + l, col] = whp[j - i, k] * Wx[k, l]
                lhsT_p[64 + k, col] = whp[j + 1, k]

    whB_rep = np.zeros((128, 1), np.float64)
    for h in range(2):
        for k in range(K):
            whB_rep[64 * h + k, :] = whp[8, k]

    bias_sb = np.zeros((128, 4), np.float64)
    for g in range(4):
        for j in range(B):
            for kk in range(16):
                bias_sb[16 * j + kk, g] = s_steady[16 * g + kk]

    cb = lhsT_p

    # cf packs everything latency-critical into one small f32 DMA:
    # cols 0:4 bias_sb, col 4 whB, col 5 winit (core-0 value; zeroed for
    # other cores in _host_prep), cols 6:38 lhsT_dh bf16 pairs bitcast.
    cf = np.zeros((128, 38), np.float32)
    cf[:, 0:4] = bias_sb
    cf[:, 4:5] = whB_rep
    cf[0:64, 5] = (vars0 - s_steady).astype(np.float32)
    dh_bf = np.ascontiguousarray(lhsT_dh.astype(BF16))   # [64, 64]
    cf[0:64, 6:38] = dh_bf.view(np.float32)

    return {
        "constb": cb.astype(BF16),
        "constf": cf,
    }


def _pack_half(x2, core, h):
    start = core * 65536 + h * HALF
    if core == 0 and h == 0:
        rows = x2[0:TT]
    else:
        rows = x2[start - W:start + HALF]
    return rows.reshape(NB, B, NJ).transpose(1, 2, 0).reshape(64, NB)


def _host_prep(series, vars0, bias, Wx, Wh):
    series = np.asarray(series, dtype=np.float32)
    x2 = (series.astype(np.float64) ** 2).astype(BF16)
    wt = _weights_host(
        np.asarray(vars0, np.float32), np.asarray(bias, np.float32),
        np.asarray(Wx, np.float32), np.asarray(Wh, np.float32),
    )
    in_maps = []
    for i in range(NCORES):
        m = dict(wt)
        m["xab"] = np.ascontiguousarray(np.concatenate(
            [_pack_half(x2, i, 0), _pack_half(x2, i, 1)], axis=1))
        if i != 0:
            cf = wt["constf"].copy()
            cf[:, 5] = 0.0
            m["constf"] = cf
        in_maps.append(m)
    return in_maps


def _assemble(results):
    hist = np.empty((T, K), dtype=np.float32)
    for i in range(NCORES):
        vout = results[i]["vout"].astype(np.float32)
        for h in range(2):
            for g in range(4):
                r = h * 4 + g
                reg = vout[:, r * NB:(r + 1) * NB]
                arr = reg.reshape(8, 16, NB).transpose(2, 0, 1).reshape(TT, 16)
                q0 = 0 if (i == 0 and h == 0) else W
                start = i * 65536 + h * HALF
                hist[start:start + HALF, 16 * g:16 * g + 16] = arr[q0:q0 + HALF]
    return hist


# ---------------------------------------------------------------------------
# numpy emulator
# ---------------------------------------------------------------------------

def emulate(inputs):
    in_maps = _host_prep(
        inputs["series"], inputs["vars0"], inputs["bias"],
        inputs["Wx"], inputs["Wh"],
    )
    results = []
    for m in in_maps:
        cb = m["constb"].astype(np.float32)
        lhsT_p = cb[:, 0:512]
        cf = m["constf"]
        lhsT_dh = np.ascontiguousarray(cf[0:64, 6:38]).view(BF16).astype(
            np.float32)
        bias_sb = cf[:, 0:4]
        whB = cf[:, 4].astype(np.float32)
        winit = cf[:, 5:6].astype(BF16)

        # phase 1
        xa = m["xab"][:, 0:NB]
        xb = m["xab"][:, NB:2 * NB]
        d_all = np.empty((128, NB), np.float32)
        d_all[0:64] = lhsT_dh.T @ xa.astype(np.float32)
        d_all[64:128] = lhsT_dh.T @ xb.astype(np.float32)

        # scan: S_scan [128, 1+NB], col 0 = winit, col 1+b = S_b (bf16)
        S_scan = np.empty((128, 1 + NB), BF16)
        S_scan[:, 0] = winit[:, 0]
        for c0 in range(0, NB, 512):
            c1 = min(c0 + 512, NB)
            st = S_scan[:, c0].astype(np.float32)
            for b in range(c0, c1):
                st = whB * st + d_all[:, b]
                S_scan[:, 1 + b] = st.astype(BF16)

        # tiles: rows 0..64 x2, rows 64..128 = S_{b-1} = S_scan cols 0..NB
        tiles = [np.zeros((128, NB), BF16), np.zeros((128, NB), BF16)]
        tiles[0][0:64] = xa
        tiles[1][0:64] = xb
        tiles[0][64:128] = S_scan[0:64, 0:NB]
        tiles[1][64:128] = S_scan[64:128, 0:NB]

        vout = np.empty((128, 8 * NB), BF16)
        for h in range(2):
            tf = tiles[h].astype(np.float32)
            for g in range(4):
                r = h * 4 + g
                ps = lhsT_p[:, 128 * g:128 * g + 128].T @ tf
                vout[:, r * NB:(r + 1) * NB] = (
                    ps + bias_sb[:, g:g + 1]
                ).astype(BF16)
        results.append({"vout": vout})
    return _assemble(results)


# ---------------------------------------------------------------------------
# Bass kernel
# ---------------------------------------------------------------------------

def _build_nc():
    import concourse.bacc as bacc
    import concourse.mybir as mybir
    import concourse.tile as tile

    f32 = mybir.dt.float32
    bf16 = mybir.dt.bfloat16

    nc = bacc.Bacc(None, target_bir_lowering=False)
    xab_d = nc.dram_tensor("xab", [64, 2 * NB], bf16, kind="ExternalInput")
    cb_d = nc.dram_tensor("constb", [128, 512], bf16, kind="ExternalInput")
    cf_d = nc.dram_tensor("constf", [128, 38], f32, kind="ExternalInput")
    vout_d = nc.dram_tensor("vout", [128, 8 * NB], bf16, kind="ExternalOutput")

    # which of the 8 per-group copies run on DVE (rest on ACT)
    DVE_IDX = [(0, 4), (0, 3, 6), (0, 4), (0, 3, 6), (0, 2, 4, 6)]
    NWARM = 6             # PE warm-up matmuls (HAM un-throttle)

    with tile.TileContext(nc) as tc:
        with (
            tc.tile_pool(name="const", bufs=1) as cpool,
            tc.tile_pool(name="xbuf", bufs=1) as xpool,
            tc.tile_pool(name="sbuf_s", bufs=1) as spool,
            tc.tile_pool(name="stage", bufs=1) as stpool,
        ):
            # constants on the scalar HWDGE queue: the small cf (with
            # bias/whB/winit/lhsT_dh packed) first, then lhsT_p
            cf_sb = cpool.tile([128, 38], f32)
            nc.scalar.dma_start(cf_sb[:], cf_d[:])
            cb_sb = cpool.tile([128, 512], bf16)
            nc.scalar.dma_start(cb_sb[:], cb_d[:])
            S_scan = spool.tile([128, 1 + NB], bf16)

            # input pieces on sync; both halves merged per piece (3D AP)
            tAB = xpool.tile([128, 2 * NB], bf16)
            x_src = xab_d[:, :].rearrange("p (t n) -> p t n", t=2)
            x_dst = tAB[0:64, :].rearrange("p (t n) -> p t n", t=2)
            for p0, p1 in IPIECES:
                nc.sync.dma_start(x_dst[:, :, p0:p1], x_src[:, :, p0:p1])

            lhsT_p = cb_sb[:, 0:512]
            lhsT_dh = cf_sb[0:64, 6:38].bitcast(bf16)
            bias_sb = cf_sb[:, 0:4]
            whB_col = cf_sb[:, 4:5]

            whB_wide = spool.tile([128, SCW], f32)
            nc.vector.memset(whB_wide[:], 1.0)
            nc.vector.tensor_scalar(
                whB_wide[:], whB_wide[:], whB_col, None,
                mybir.AluOpType.mult,
            )
            # scan initial column: cast winit (cf col 5) to bf16 in place
            nc.vector.tensor_copy(S_scan[:, 0:1], cf_sb[:, 5:6])

            staged = stpool.tile([128, 8 * NB], bf16, name="staged")
            st_dst = vout_d[:, :].rearrange("p (r n) -> p r n", r=8)
            st_src = staged[:, :].rearrange("p (r n) -> p r n", r=8)

            # tile for PE warm-up matmuls (contents irrelevant; memset so
            # CoreSim doesn't flag an uninitialized read)
            warm_sb = spool.tile([64, 576], bf16, name="warm_sb")
            nc.vector.memset(warm_sb[:], 0.0)

            with (
                tc.tile_pool(name="dps", bufs=1, space="PSUM") as dps,
                tc.tile_pool(name="pps", bufs=1, space="PSUM") as pps,
            ):
                # PE warm-up: matmuls with no input dependencies so the
                # HAM un-throttles (1.2 -> 2.4 GHz) before real work.
                # Alternating PSUM tags keep them back-to-back (no WAW).
                for wi in range(NWARM):
                    w_ps = dps.tile([128, SCW], f32, tag=f"d{wi % 2}",
                                    name=f"warm{wi}")
                    nc.tensor.matmul(
                        w_ps[0:64, 0:512], warm_sb[0:64, 512:576],
                        warm_sb[0:64, 0:512],
                        start=True, stop=True, tile_position=(0, 0),
                    )

                def emit_d(c):
                    c0, c1 = SCHUNKS[c]
                    n = c1 - c0
                    d_ps = dps.tile([128, SCW], f32, tag=f"d{c % 2}",
                                    name=f"dpsx{c}")
                    nc.tensor.matmul(
                        d_ps[0:64, 0:n], lhsT_dh, tAB[0:64, c0:c1],
                        start=True, stop=True, tile_position=(0, 0),
                    )
                    nc.tensor.matmul(
                        d_ps[64:128, 0:n], lhsT_dh,
                        tAB[0:64, NB + c0:NB + c1],
                        start=True, stop=True, tile_position=(0, 64),
                    )
                    return d_ps

                def emit_scan(c, d_ps):
                    c0, c1 = SCHUNKS[c]
                    n = c1 - c0
                    nc.vector.tensor_tensor_scan(
                        S_scan[:, 1 + c0:1 + c1],
                        whB_wide[:, 0:n],
                        d_ps[:, 0:n],
                        S_scan[:, c0:c0 + 1],
                        mybir.AluOpType.mult,
                        mybir.AluOpType.add,
                    )
                    if c1 <= CGROUPS[0][1]:
                        # group 0: chunk-level carry copies on the idle
                        # scalar ring so its first matmul isn't gated on
                        # the whole group's scan chain
                        nc.scalar.dma_start(tAB[64:128, c0:c1],
                                            S_scan[0:64, c0:c1])
                        nc.scalar.dma_start(tAB[64:128, NB + c0:NB + c1],
                                            S_scan[64:128, c0:c1])


                # chunks belonging to each copy group
                def chunks_of(gi):
                    g0, g1 = CGROUPS[gi]
                    return [(c, s) for c, s in enumerate(SCHUNKS)
                            if g0 <= s[0] < g1]

                d_tiles = {0: emit_d(0), 1: emit_d(1)}
                nd = 2
                np_ps = 0
                nio = 0
                for gi, (g0, g1) in enumerate(CGROUPS):
                    gchunks = chunks_of(gi)
                    for c, _ in gchunks:
                        emit_scan(c, d_tiles.pop(c))
                        if nd < len(SCHUNKS):
                            d_tiles[nd] = emit_d(nd)
                            nd += 1
                    # carry copies for the group (SBUF->SBUF).  Group 0
                    # is handled per-chunk inside emit_scan; group 1
                    # rides the idle scalar HWDGE ring (low latency);
                    # late ones go to gpsimd so they don't queue behind
                    # vout DMAs on scalar.
                    if gi > 0:
                        enga = nc.scalar if gi < 2 else nc.gpsimd
                        enga.dma_start(tAB[64:128, g0:g1],
                                       S_scan[0:64, g0:g1])
                        enga.dma_start(tAB[64:128, NB + g0:NB + g1],
                                       S_scan[64:128, g0:g1])
                    w = g1 - g0
                    # matmul column spans = the group's scan chunks
                    spans = [(c0 - g0, c1 - c0) for _, (c0, c1) in gchunks]
                    for ri in range(8):
                        h, g = divmod(ri, 4)
                        r = h * 4 + g
                        p_ps = pps.tile([128, 1024], f32,
                                        tag=f"p{np_ps % 3}")
                        np_ps += 1
                        for o, n in spans:
                            nc.tensor.matmul(
                                p_ps[:, o:o + n],
                                lhsT_p[:, 128 * g:128 * g + 128],
                                tAB[:, h * NB + g0 + o:h * NB + g0 + o + n],
                                start=True, stop=True,
                                tile_position=(0, 0),
                            )
                        so = r * NB
                        if ri in DVE_IDX[gi]:
                            nc.vector.tensor_scalar(
                                staged[:, so + g0:so + g1], p_ps[:, 0:w],
                                1.0,
                                bias_sb[:, g:g + 1],
                                mybir.AluOpType.mult,
                                mybir.AluOpType.add,
                            )
                        else:
                            nc.scalar.activation(
                                staged[:, so + g0:so + g1], p_ps[:, 0:w],
                                mybir.ActivationFunctionType.Identity,
                                bias=bias_sb[:, g:g + 1],
                            )
                        last = gi == len(CGROUPS) - 1
                        if (not last and ri in (3, 7)) or (last and ri % 2):
                            # merged vout DMA; finer in the last group and
                            # on the HWDGE rings so the SWDGE drain does
                            # not dominate teardown
                            rj = ri - 1 if last else ri - 3
                            if last:
                                eng = nc.sync if nio % 2 == 0 else nc.scalar
                            else:
                                eng = nc.sync if nio % 2 == 0 else nc.gpsimd
                            eng.dma_start(
                                st_dst[:, rj:ri + 1, g0:g1],
                                st_src[:, rj:ri + 1, g0:g1],
                            )
                            nio += 1

    nc.compile()
    return nc


def run(inputs, trace=False, **kw):
    from concourse.bass_utils import run_bass_kernel_spmd

    if "nc" not in _CACHE:
        _CACHE["nc"] = _build_nc()
    nc = _CACHE["nc"]
    in_maps = _host_prep(
        inputs["series"], inputs["vars0"], inputs["bias"],
        inputs["Wx"], inputs["Wh"],
    )
    res = run_bass_kernel_spmd(
        nc, in_maps, core_ids=list(range(NCORES)), trace=trace, **kw
    )
    return _assemble(res.results), res


def kernel(series, vars0, bias, Wx, Wh):
    out, _ = run(
        {"series": series, "vars0": vars0, "bias": bias, "Wx": Wx, "Wh": Wh}
    )
    return out



# revision 57
# speedup vs baseline: 1.0263x; 1.0263x over previous
"""MixGARCH Trainium2 kernel, v10 (final): B=8 block-scan, pipelined
fine-grained phase 2, DMA-instruction-lean (~56.2us vs 64.6us baseline).

Math (unchanged since v3): subtract the steady state s=(bias+1e-6)/(1-Wh)
so the recurrence is linear with no bias (ReLU is a no-op: all terms are
non-negative).  Per half (32768 steps + 256 warm-up): phase-1 matmuls
reduce each 8-step block to d_b = sum_i Wh^(7-i) Wx x2; a hardware
tensor_tensor_scan chains S_b = Wh^8 S_{b-1} + d_b for both halves at
once (128 partitions); phase-2 matmuls expand every block to its 8
outputs with the carry S_{b-1} folded into the same 128-deep contraction
(rhs rows 0:64 = packed x2, rows 64:128 = carry, placed by an SBUF->SBUF
partition-shift DMA).

Scheduling (what the traces drove):
  - every dma_start costs ~0.6us of issuing-engine time and ~2us of
    serialized ring completion latency, so DMA instructions are few and
    routed: cf (with winit + lhsT_dh bitcast-packed) then lhsT_p on the
    scalar ring; 6 merged input pieces (both halves via 3D rearrange
    APs) on sync; early-group S-copies on scalar, late on gpsimd; vout
    staged in one [128, 8*NB] tile and shipped as 2 merged 4-region
    DMAs per copy group (sync/gpsimd), 4 finer HWDGE DMAs in the last.
  - scan chunks [256,256,512x7,32]; phase-2 emitted per copy group
    ([512,1024,1024,1024,544]) region-major through 3 rotating PSUM
    tiles; PSUM->SBUF cast+bias copies split DVE/ACT to balance with
    the scan (~26us busy each, just under the ~27us HBM-DMA floor).
"""

import numpy as np
import ml_dtypes

BF16 = ml_dtypes.bfloat16

T = 524288
K = 64
NJ = 8
NCORES = 8
W = 256               # warmup steps per half
HALF = 32768
TT = W + HALF         # 33024 steps per half
B = 8                 # block size
NB = TT // B          # 4128 blocks per half
SCW = 512             # whB_wide width (max scan chunk)
# scan chunks (also d-matmul chunks)
SCHUNKS = [(0, 256), (256, 512), (512, 1024), (1024, 1536), (1536, 2048),
           (2048, 2560), (2560, 3072), (3072, 3584), (3584, 4096),
           (4096, 4128)]
# copy groups: spans of scan chunks that share one PSUM tile / copy / DMA
CGROUPS = [(0, 512), (512, 1536), (1536, 2560), (2560, 3584), (3584, 4128)]
# input DMA pieces (merged across both halves via 3D AP)
IPIECES = [(0, 256), (256, 512), (512, 1536), (1536, 2560), (2560, 3584),
           (3584, 4128)]

_CACHE = {}


def _weights_host(vars0, bias, Wx, Wh):
    Wx = Wx.astype(np.float64)
    Wh = Wh.astype(np.float64)
    bias = bias.astype(np.float64)
    s_steady = (bias + 1e-6) / (1.0 - Wh)
    whp = Wh[None, :] ** np.arange(10)[:, None]   # whp[e, k]
    vars0 = np.asarray(vars0, np.float64)

    # d/S layout partition: 64h + k.  xin rows (per half tile): 8i + l.
    # phase-2 out partition: 16j + kk (k = 16g + kk).
    lhsT_dh = np.zeros((64, 64), np.float64)
    for i in range(B):
        for l in range(NJ):
            for k in range(K):
                lhsT_dh[8 * i + l, k] = whp[7 - i, k] * Wx[k, l]

    # lhsT_p per g: [128, 128]; rows 0..64 intra-block, rows 64..128 carry.
    lhsT_p = np.zeros((128, 4 * 128), np.float64)
    for g in range(4):
        for j in range(B):
            for kk in range(16):
                k = 16 * g + kk
                col = 128 * g + 16 * j + kk
                for i in range(j + 1):
                    for l in range(NJ):
                        lhsT_p[8 * i + l, col] = whp[j - i, k] * Wx[k, l]
                lhsT_p[64 + k, col] = whp[j + 1, k]

    whB_rep = np.zeros((128, 1), np.float64)
    for h in range(2):
        for k in range(K):
            whB_rep[64 * h + k, :] = whp[8, k]

    bias_sb = np.zeros((128, 4), np.float64)
    for g in range(4):
        for j in range(B):
            for kk in range(16):
                bias_sb[16 * j + kk, g] = s_steady[16 * g + kk]

    cb = lhsT_p

    # cf packs everything latency-critical into one small f32 DMA:
    # cols 0:4 bias_sb, col 4 whB, col 5 winit (core-0 value; zeroed for
    # other cores in _host_prep), cols 6:38 lhsT_dh bf16 pairs bitcast.
    cf = np.zeros((128, 38), np.float32)
    cf[:, 0:4] = bias_sb
    cf[:, 4:5] = whB_rep
    cf[0:64, 5] = (vars0 - s_steady).astype(np.float32)
    dh_bf = np.ascontiguousarray(lhsT_dh.astype(BF16))   # [64, 64]
    cf[0:64, 6:38] = dh_bf.view(np.float32)

    return {
        "constb": cb.astype(BF16),
        "constf": cf,
    }


def _pack_half(x2, core, h):
    start = core * 65536 + h * HALF
    if core == 0 and h == 0:
        rows = x2[0:TT]
    else:
        rows = x2[start - W:start + HALF]
    return rows.reshape(NB, B, NJ).transpose(1, 2, 0).reshape(64, NB)


def _host_prep(series, vars0, bias, Wx, Wh):
    series = np.asarray(series, dtype=np.float32)
    x2 = (series.astype(np.float64) ** 2).astype(BF16)
    wt = _weights_host(
        np.asarray(vars0, np.float32), np.asarray(bias, np.float32),
        np.asarray(Wx, np.float32), np.asarray(Wh, np.float32),
    )
    in_maps = []
    for i in range(NCORES):
        m = dict(wt)
        m["xab"] = np.ascontiguousarray(np.concatenate(
            [_pack_half(x2, i, 0), _pack_half(x2, i, 1)], axis=1))
        if i != 0:
            cf = wt["constf"].copy()
            cf[:, 5] = 0.0
            m["constf"] = cf
        in_maps.append(m)
    return in_maps


def _assemble(results):
    hist = np.empty((T, K), dtype=np.float32)
    for i in range(NCORES):
        vout = results[i]["vout"].astype(np.float32)
        for h in range(2):
            for g in range(4):
                r = h * 4 + g
                reg = vout[:, r * NB:(r + 1) * NB]
                arr = reg.reshape(8, 16, NB).transpose(2, 0, 1).reshape(TT, 16)
                q0 = 0 if (i == 0 and h == 0) else W
                start = i * 65536 + h * HALF
                hist[start:start + HALF, 16 * g:16 * g + 16] = arr[q0:q0 + HALF]
    return hist


# ---------------------------------------------------------------------------
# numpy emulator
# ---------------------------------------------------------------------------

def emulate(inputs):
    in_maps = _host_prep(
        inputs["series"], inputs["vars0"], inputs["bias"],
        inputs["Wx"], inputs["Wh"],
    )
    results = []
    for m in in_maps:
        cb = m["constb"].astype(np.float32)
        lhsT_p = cb[:, 0:512]
        cf = m["constf"]
        lhsT_dh = np.ascontiguousarray(cf[0:64, 6:38]).view(BF16).astype(
            np.float32)
        bias_sb = cf[:, 0:4]
        whB = cf[:, 4].astype(np.float32)
        winit = cf[:, 5:6].astype(BF16)

        # phase 1
        xa = m["xab"][:, 0:NB]
        xb = m["xab"][:, NB:2 * NB]
        d_all = np.empty((128, NB), np.float32)
        d_all[0:64] = lhsT_dh.T @ xa.astype(np.float32)
        d_all[64:128] = lhsT_dh.T @ xb.astype(np.float32)

        # scan: S_scan [128, 1+NB], col 0 = winit, col 1+b = S_b (bf16)
        S_scan = np.empty((128, 1 + NB), BF16)
        S_scan[:, 0] = winit[:, 0]
        for c0 in range(0, NB, 512):
            c1 = min(c0 + 512, NB)
            st = S_scan[:, c0].astype(np.float32)
            for b in range(c0, c1):
                st = whB * st + d_all[:, b]
                S_scan[:, 1 + b] = st.astype(BF16)

        # tiles: rows 0..64 x2, rows 64..128 = S_{b-1} = S_scan cols 0..NB
        tiles = [np.zeros((128, NB), BF16), np.zeros((128, NB), BF16)]
        tiles[0][0:64] = xa
        tiles[1][0:64] = xb
        tiles[0][64:128] = S_scan[0:64, 0:NB]
        tiles[1][64:128] = S_scan[64:128, 0:NB]

        vout = np.empty((128, 8 * NB), BF16)
        for h in range(2):
            tf = tiles[h].astype(np.float32)
            for g in range(4):
                r = h * 4 + g
                ps = lhsT_p[:, 128 * g:128 * g + 128].T @ tf
                vout[:, r * NB:(r + 1) * NB] = (
                    ps + bias_sb[:, g:g + 1]
                ).astype(BF16)
        results.append({"vout": vout})
    return _assemble(results)


# ---------------------------------------------------------------------------
# Bass kernel
# ---------------------------------------------------------------------------

def _build_nc():
    import concourse.bacc as bacc
    import concourse.mybir as mybir
    import concourse.tile as tile

    f32 = mybir.dt.float32
    bf16 = mybir.dt.bfloat16

    nc = bacc.Bacc(None, target_bir_lowering=False)
    xab_d = nc.dram_tensor("xab", [64, 2 * NB], bf16, kind="ExternalInput")
    cb_d = nc.dram_tensor("constb", [128, 512], bf16, kind="ExternalInput")
    cf_d = nc.dram_tensor("constf", [128, 38], f32, kind="ExternalInput")
    vout_d = nc.dram_tensor("vout", [128, 8 * NB], bf16, kind="ExternalOutput")

    # which of the 8 per-group copies run on DVE (rest on ACT)
    DVE_IDX = [(0, 4), (0, 3, 6), (0, 4), (0, 3, 6), (0, 2, 4, 6)]
    NWARM = 6             # PE warm-up matmuls (HAM un-throttle)

    with tile.TileContext(nc) as tc:
        with (
            tc.tile_pool(name="const", bufs=1) as cpool,
            tc.tile_pool(name="xbuf", bufs=1) as xpool,
            tc.tile_pool(name="sbuf_s", bufs=1) as spool,
            tc.tile_pool(name="stage", bufs=1) as stpool,
        ):
            # constants on the scalar HWDGE queue: the small cf (with
            # bias/whB/winit/lhsT_dh packed) first, then lhsT_p
            cf_sb = cpool.tile([128, 38], f32)
            nc.scalar.dma_start(cf_sb[:], cf_d[:])
            cb_sb = cpool.tile([128, 512], bf16)
            nc.scalar.dma_start(cb_sb[:], cb_d[:])
            S_scan = spool.tile([128, 1 + NB], bf16)

            # input pieces on sync; both halves merged per piece (3D AP)
            tAB = xpool.tile([128, 2 * NB], bf16)
            x_src = xab_d[:, :].rearrange("p (t n) -> p t n", t=2)
            x_dst = tAB[0:64, :].rearrange("p (t n) -> p t n", t=2)
            for p0, p1 in IPIECES:
                nc.sync.dma_start(x_dst[:, :, p0:p1], x_src[:, :, p0:p1])

            lhsT_p = cb_sb[:, 0:512]
            lhsT_dh = cf_sb[0:64, 6:38].bitcast(bf16)
            bias_sb = cf_sb[:, 0:4]
            whB_col = cf_sb[:, 4:5]

            whB_wide = spool.tile([128, SCW], f32)
            nc.vector.memset(whB_wide[:], 1.0)
            nc.vector.tensor_scalar(
                whB_wide[:], whB_wide[:], whB_col, None,
                mybir.AluOpType.mult,
            )
            # scan initial column: cast winit (cf col 5) to bf16 in place
            nc.vector.tensor_copy(S_scan[:, 0:1], cf_sb[:, 5:6])

            staged = stpool.tile([128, 8 * NB], bf16, name="staged")
            st_dst = vout_d[:, :].rearrange("p (r n) -> p r n", r=8)
            st_src = staged[:, :].rearrange("p (r n) -> p r n", r=8)

            # tile for PE warm-up matmuls (contents irrelevant; memset so
            # CoreSim doesn't flag an uninitialized read)
            warm_sb = spool.tile([64, 576], bf16, name="warm_sb")
            nc.vector.memset(warm_sb[:], 0.0)

            with (
                tc.tile_pool(name="dps", bufs=1, space="PSUM") as dps,
                tc.tile_pool(name="pps", bufs=1, space="PSUM") as pps,
            ):
                # PE warm-up: matmuls with no input dependencies so the
                # HAM un-throttles (1.2 -> 2.4 GHz) before real work.
                # Alternating PSUM tags keep them back-to-back (no WAW).
                for wi in range(NWARM):
                    w_ps = dps.tile([128, SCW], f32, tag=f"d{wi % 2}",
                                    name=f"warm{wi}")
                    nc.tensor.matmul(
                        w_ps[0:64, 0:512], warm_sb[0:64, 512:576],
                        warm_sb[0:64, 0:512],
                        start=True, stop=True, tile_position=(0, 0),
                    )

                def emit_d(c):
                    c0, c1 = SCHUNKS[c]
                    n = c1 - c0
                    d_ps = dps.tile([128, SCW], f32, tag=f"d{c % 2}",
                                    name=f"dpsx{c}")
                    nc.tensor.matmul(
                        d_ps[0:64, 0:n], lhsT_dh, tAB[0:64, c0:c1],
                        start=True, stop=True, tile_position=(0, 0),
                    )
                    nc.tensor.matmul(
                        d_ps[64:128, 0:n], lhsT_dh,
                        tAB[0:64, NB + c0:NB + c1],
                        start=True, stop=True, tile_position=(0, 64),
                    )
                    return d_ps

                def emit_scan(c, d_ps):
                    c0, c1 = SCHUNKS[c]
                    n = c1 - c0
                    nc.vector.tensor_tensor_scan(
                        S_scan[:, 1 + c0:1 + c1],
                        whB_wide[:, 0:n],
                        d_ps[:, 0:n],
                        S_scan[:, c0:c0 + 1],
                        mybir.AluOpType.mult,
                        mybir.AluOpType.add,
                    )
                    if c1 <= CGROUPS[0][1]:
                        # group 0: chunk-level carry copies on the idle
                        # scalar ring so its first matmul isn't gated on
                        # the whole group's scan chain
                        nc.scalar.dma_start(tAB[64:128, c0:c1],
                                            S_scan[0:64, c0:c1])
                        nc.scalar.dma_start(tAB[64:128, NB + c0:NB + c1],
                                            S_scan[64:128, c0:c1])


                # chunks belonging to each copy group
                def chunks_of(gi):
                    g0, g1 = CGROUPS[gi]
                    return [(c, s) for c, s in enumerate(SCHUNKS)
                            if g0 <= s[0] < g1]

                d_tiles = {0: emit_d(0), 1: emit_d(1)}
                nd = 2
                np_ps = 0
                nio = 0
                for gi, (g0, g1) in enumerate(CGROUPS):
                    gchunks = chunks_of(gi)
                    for c, _ in gchunks:
                        emit_scan(c, d_tiles.pop(c))
                        if nd < len(SCHUNKS):
                            d_tiles[nd] = emit_d(nd)
                            nd += 1
                    # carry copies for the group (SBUF->SBUF).  Group 0
                    # is handled per-chunk inside emit_scan; group 1
                    # rides the idle scalar HWDGE ring (low latency);
                    # late ones go to gpsimd so they don't queue behind
                    # vout DMAs on scalar.
                    if gi > 0:
                        enga = nc.scalar if gi < 2 else nc.gpsimd
                        enga.dma_start(tAB[64:128, g0:g1],
                                       S_scan[0:64, g0:g1])
                        enga.dma_start(tAB[64:128, NB + g0:NB + g1],
                                       S_scan[64:128, g0:g1])
                    w = g1 - g0
                    # matmul column spans = the group's scan chunks
                    spans = [(c0 - g0, c1 - c0) for _, (c0, c1) in gchunks]
                    for ri in range(8):
                        h, g = divmod(ri, 4)
                        r = h * 4 + g
                        p_ps = pps.tile([128, 1024], f32,
                                        tag=f"p{np_ps % 3}")
                        np_ps += 1
                        for o, n in spans:
                            nc.tensor.matmul(
                                p_ps[:, o:o + n],
                                lhsT_p[:, 128 * g:128 * g + 128],
                                tAB[:, h * NB + g0 + o:h * NB + g0 + o + n],
                                start=True, stop=True,
                                tile_position=(0, 0),
                            )
                        so = r * NB
                        if ri in DVE_IDX[gi]:
                            nc.vector.tensor_scalar(
                                staged[:, so + g0:so + g1], p_ps[:, 0:w],
                                1.0,
                                bias_sb[:, g:g + 1],
                                mybir.AluOpType.mult,
                                mybir.AluOpType.add,
                            )
                        else:
                            nc.scalar.activation(
                                staged[:, so + g0:so + g1], p_ps[:, 0:w],
                                mybir.ActivationFunctionType.Identity,
                                bias=bias_sb[:, g:g + 1],
                            )
                        last = gi == len(CGROUPS) - 1
                        if ri % 2:
                            # 2-region vout DMAs: ship bytes as soon as
                            # they exist; last group on the HWDGE rings
                            # so the SWDGE drain does not dominate
                            # teardown
                            if last:
                                eng = nc.sync if nio % 2 == 0 else nc.scalar
                            else:
                                eng = nc.sync if nio % 2 == 0 else nc.gpsimd
                            eng.dma_start(
                                st_dst[:, ri - 1:ri + 1, g0:g1],
                                st_src[:, ri - 1:ri + 1, g0:g1],
                            )
                            nio += 1

    nc.compile()
    return nc


def run(inputs, trace=False, **kw):
    from concourse.bass_utils import run_bass_kernel_spmd

    if "nc" not in _CACHE:
        _CACHE["nc"] = _build_nc()
    nc = _CACHE["nc"]
    in_maps = _host_prep(
        inputs["series"], inputs["vars0"], inputs["bias"],
        inputs["Wx"], inputs["Wh"],
    )
    res = run_bass_kernel_spmd(
        nc, in_maps, core_ids=list(range(NCORES)), trace=trace, **kw
    )
    return _assemble(res.results), res


def kernel(series, vars0, bias, Wx, Wh):
    out, _ = run(
        {"series": series, "vars0": vars0, "bias": bias, "Wx": Wx, "Wh": Wh}
    )
    return out

